# revision 1
# baseline (speedup 1.0000x reference)
"""Trainium2 Bass kernel for GroupedQueryAttention (sparse sliding-window + global).

Sharding: 8 cores = 2 (batch) x 4 (GQA groups). Core c handles batch c//4 and
kv-head g=c%4 together with its 4 query heads (heads 4g..4g+3). Wq/Wk/Wv are
column-sharded, Wo row-sharded; each core emits a transposed partial output
outT = (context_g @ Wo_g)^T which the host transposes and sums per batch.
"""

import sys

for _p in (
    "/opt/trn_rl_repo",
    "/root/.axon_site",
    "/root/.axon_site/_ro/pypackages",
    "/root/.axon_site/_ro/trn_rl_repo",
):
    if _p not in sys.path:
        sys.path.insert(0, _p)

from contextlib import ExitStack

import numpy as np

import concourse.bass as bass  # noqa: F401  (registers engine classes)
import concourse.tile as tile
from concourse import bacc, mybir
from concourse.bass_utils import run_bass_kernel_spmd
from concourse.masks import make_identity

B, S, DM = 2, 2048, 1024
NH, NKV, DH = 16, 4, 64
HPC = 4  # q heads per core (one full GQA group)
WINDOW, NGLOB = 256, 4
SCALE = 1.0 / np.sqrt(DH)
CAP = 15.0
EPS = 1e-8
P = 128
NT = S // P  # 16 sequence tiles
G = HPC + 1  # 4 q heads + 1 k head share L2norm/RoPE processing
F32 = mybir.dt.float32
F32R = mybir.dt.float32r
BF16 = mybir.dt.bfloat16
MULT = mybir.AluOpType.mult


def _build_kernel(ctx, tc, d):
    nc = tc.nc

    consts = ctx.enter_context(tc.tile_pool(name="consts", bufs=1))
    ident = consts.tile([P, P], F32)
    make_identity(nc, ident[:])
    ident_bf = consts.tile([P, P], BF16)
    nc.vector.tensor_copy(ident_bf[:], ident[:])

    wqkv_sb = consts.tile([P, 8, 384], BF16)
    nc.sync.dma_start(wqkv_sb[:], d["wqkv"].rearrange("(c p) n -> p c n", p=P))
    wo_sb = consts.tile([P, 2, DM], BF16)
    nc.sync.dma_start(wo_sb[:], d["wo"].rearrange("(c p) n -> p c n", p=P))
    cos_sb = consts.tile([P, NT, 32], F32)
    nc.sync.dma_start(cos_sb[:], d["cos"].rearrange("(t p) n -> p t n", p=P))
    sin_sb = consts.tile([P, NT, 32], F32)
    nc.sync.dma_start(sin_sb[:], d["sin"].rearrange("(t p) n -> p t n", p=P))
    ones1 = consts.tile([P, 1], F32)
    nc.vector.memset(ones1[:], 1.0)

    # persistent per-s-chunk tensors
    qt_pool = ctx.enter_context(tc.tile_pool(name="qt", bufs=NT))
    kt_pool = ctx.enter_context(tc.tile_pool(name="kt", bufs=NT))
    v_pool = ctx.enter_context(tc.tile_pool(name="v", bufs=NT))
    ctx_pool = ctx.enter_context(tc.tile_pool(name="ctx", bufs=8))

    xp = ctx.enter_context(tc.tile_pool(name="xp", bufs=3))
    xtp = ctx.enter_context(tc.tile_pool(name="xtp", bufs=10))
    work = ctx.enter_context(tc.tile_pool(name="work", bufs=3))
    attn = ctx.enter_context(tc.tile_pool(name="attn", bufs=3))

    ps_t = ctx.enter_context(tc.tile_pool(name="ps_t", bufs=2, space="PSUM"))
    ps_mm = ctx.enter_context(tc.tile_pool(name="ps_mm", bufs=2, space="PSUM"))
    ps_sc = ctx.enter_context(tc.tile_pool(name="ps_sc", bufs=2, space="PSUM"))
    ps_cx = ctx.enter_context(tc.tile_pool(name="ps_cx", bufs=2, space="PSUM"))

    qtiles, ktiles, vtiles = [], [], []
    ctxt = [[None] * 4, [None] * 4]
    for c in range(2):
        for sc in range(4):
            ctile = ctx_pool.tile([P, 512], BF16, name=f"ctx_{c}_{sc}", tag="ctx")
            ctxt[c][sc] = ctile

    # ---------------- Phase A: QKV projection, L2 norm, RoPE, transposes ----
    for i in range(NT):
        x_sb = xp.tile([P, DM], F32, tag="x")
        nc.sync.dma_start(x_sb[:], d["xs"][P * i : P * (i + 1), :])
        xb = xp.tile([P, DM], BF16, tag="xb")
        nc.vector.tensor_copy(xb[:], x_sb[:])

        xts = []
        for mj in range(8):
            pt = ps_t.tile([P, P], BF16, name=f"ptx_{i}_{mj}", tag="t")
            nc.tensor.transpose(pt[:], xb[:, P * mj : P * (mj + 1)], ident_bf[:])
            xt = xtp.tile([P, P], BF16, name=f"xt_{i}_{mj}", tag="xt")
            if mj % 2 == 0:
                nc.scalar.copy(xt[:], pt[:])
            else:
                nc.vector.tensor_copy(xt[:], pt[:])
            xts.append(xt)

        pq = ps_mm.tile([P, 384], F32, name=f"pqkv_{i}", tag="mm")
        for mj in range(8):
            nc.tensor.matmul(
                pq[:],
                lhsT=xts[mj][:],
                rhs=wqkv_sb[:, mj, :],
                start=(mj == 0),
                stop=(mj == 7),
            )

        # L2 normalization over d for q heads and k head (first 320 cols)
        ssq = work.tile([P, G * DH], F32, tag="ssq")
        nc.scalar.square(ssq[:], pq[:, 0 : G * DH])
        red = work.tile([P, G], F32, tag="red")
        nc.vector.tensor_reduce(
            red[:],
            ssq[:].rearrange("p (g n) -> p g n", g=G),
            axis=mybir.AxisListType.X,
            op=mybir.AluOpType.add,
        )
        nrm = work.tile([P, G], F32, tag="nrm")
        nc.scalar.sqrt(nrm[:], red[:])
        nrm2 = work.tile([P, G], F32, tag="nrm2")
        nc.vector.tensor_scalar_add(nrm2[:], nrm[:], EPS)
        rcn = work.tile([P, G], F32, tag="rcn")
        nc.vector.reciprocal(rcn[:], nrm2[:])
        qkn = work.tile([P, G * DH], F32, tag="qkn")
        nc.vector.tensor_tensor(
            qkn[:].rearrange("p (g n) -> p g n", g=G),
            pq[:, 0 : G * DH].rearrange("p (g n) -> p g n", g=G),
            rcn[:].unsqueeze(-1).broadcast_to([P, G, DH]),
            op=MULT,
        )

        # v (+ ones column for softmax sums)
        vt_i = v_pool.tile([P, 65], BF16, name=f"v_{i}", tag="v")
        nc.scalar.copy(vt_i[:, 64:65], ones1[:])
        nc.scalar.copy(vt_i[:, 0:64], pq[:, 320:384])
        vtiles.append(vt_i)

        # RoPE: rotate halves (d, d+32) with cos/sin of this s-chunk
        qv = qkn[:].rearrange("p (g n) -> p g n", g=G)
        x1, x2 = qv[:, :, 0:32], qv[:, :, 32:64]
        cb = cos_sb[:, i, :].unsqueeze(1).broadcast_to([P, G, 32])
        sbr = sin_sb[:, i, :].unsqueeze(1).broadcast_to([P, G, 32])
        rp = work.tile([P, G * DH], BF16, tag="rp")
        rv = rp[:].rearrange("p (g n) -> p g n", g=G)
        ta = work.tile([P, G * 32], F32, tag="ta")
        tb = work.tile([P, G * 32], F32, tag="tb")
        tav = ta[:].rearrange("p (g n) -> p g n", g=G)
        tbv = tb[:].rearrange("p (g n) -> p g n", g=G)
        nc.vector.tensor_tensor(tav, x1, cb, op=MULT)
        nc.vector.tensor_tensor(tbv, x2, sbr, op=MULT)
        nc.vector.tensor_sub(rv[:, :, 0:32], tav, tbv)
        nc.vector.tensor_tensor(tav, x1, sbr, op=MULT)
        nc.vector.tensor_tensor(tbv, x2, cb, op=MULT)
        nc.vector.tensor_add(rv[:, :, 32:64], tav, tbv)

        # transpose q (2x 128-col blocks = 4 heads) and k (64 cols)
        qt_i = qt_pool.tile([64, HPC * P], BF16, name=f"qt_{i}", tag="qt")
        for hp in range(2):
            ptq = ps_t.tile([P, P], BF16, name=f"ptq_{i}_{hp}", tag="t")
            nc.tensor.transpose(ptq[:], rp[:, P * hp : P * (hp + 1)], ident_bf[:])
            nc.scalar.copy(qt_i[:, (2 * hp) * P : (2 * hp) * P + P], ptq[0:64, :])
            nc.vector.tensor_copy(
                qt_i[:, (2 * hp + 1) * P : (2 * hp + 1) * P + P], ptq[64:128, :]
            )
        ptk = ps_t.tile([P, P], BF16, name=f"ptk_{i}", tag="t")
        nc.tensor.transpose(ptk[0:64, :], rp[:, 256:320], ident_bf[:])
        kt_i = kt_pool.tile([64, P], BF16, name=f"kt_{i}", tag="kt")
        nc.scalar.copy(kt_i[:], ptk[0:64, :])
        qtiles.append(qt_i)
        ktiles.append(kt_i)

    # ---------------- Phase B: banded attention --------------------------
    for t in range(NT):
        kts = list(range(max(0, t - 2), t + 1))
        mb = attn.tile([P, 3, P], BF16, tag="mb")
        nc.sync.dma_start(mb[:], d["band"][t])
        qrhs = qtiles[t][:].rearrange("p (h q) -> p h q", h=HPC)
        pcx = ps_cx.tile([65, 512], F32, name=f"pcx_{t}", tag="cx")

        for j_, kt in enumerate(kts):
            j = kt - (t - 2)
            ps = ps_sc.tile([P, 512], F32, name=f"psc_{t}_{kt}", tag="sc")
            nc.tensor.matmul(
                ps[:], lhsT=ktiles[kt][:], rhs=qrhs, start=True, stop=True
            )
            ex = attn.tile([P, 512], BF16, tag="ex")
            nc.scalar.activation(
                ex[:], ps[:], mybir.ActivationFunctionType.Exp, scale=SCALE
            )
            em = attn.tile([P, 512], BF16, tag="em")
            nc.vector.tensor_tensor(
                em[:].rearrange("p (h q) -> p h q", h=HPC),
                ex[:].rearrange("p (h q) -> p h q", h=HPC),
                mb[:, j, :].unsqueeze(1).broadcast_to([P, HPC, P]),
                op=MULT,
            )
            nc.tensor.matmul(
                pcx[:],
                lhsT=vtiles[kt][:],
                rhs=em[:],
                start=(j_ == 0),
                stop=(j_ == len(kts) - 1 and t < 3),
            )

        if t >= 3:
            gm = attn.tile([4, P], BF16, tag="gm")
            nc.sync.dma_start(gm[:], d["glob"][t])
            psg = ps_sc.tile([4, 512], F32, name=f"psg_{t}", tag="sc")
            nc.tensor.matmul(
                psg[:], lhsT=ktiles[0][:, 0:4], rhs=qrhs, start=True, stop=True
            )
            exg = attn.tile([4, 512], BF16, tag="exg")
            nc.scalar.activation(
                exg[:], psg[:], mybir.ActivationFunctionType.Exp, scale=SCALE
            )
            emg = attn.tile([4, 512], BF16, tag="emg")
            nc.vector.tensor_tensor(
                emg[:].rearrange("p (h q) -> p h q", h=HPC),
                exg[:].rearrange("p (h q) -> p h q", h=HPC),
                gm[:].unsqueeze(1).broadcast_to([4, HPC, P]),
                op=MULT,
            )
            nc.tensor.matmul(
                pcx[:],
                lhsT=vtiles[0][0:4, :],
                rhs=emg[:],
                start=False,
                stop=True,
            )

        # softmax denominators (row 64 of pcx) -> reciprocal -> broadcast
        sm = attn.tile([1, 512], F32, tag="sm")
        nc.scalar.copy(sm[:], pcx[64:65, :])
        rb = attn.tile([64, 512], F32, tag="rb")
        nc.gpsimd.partition_broadcast(rb[:], sm[:])
        rc = attn.tile([64, 512], F32, tag="rc")
        nc.vector.reciprocal(rc[:], rb[:])

        sc_, qoff = t // 4, (t % 4) * P
        for h in range(HPC):
            c, p0 = h // 2, 64 * (h % 2)
            nc.vector.tensor_tensor(
                ctxt[c][sc_][p0 : p0 + 64, qoff : qoff + P],
                pcx[0:64, h * P : (h + 1) * P],
                rc[:, h * P : (h + 1) * P],
                op=MULT,
            )

    # ---------------- Phase C: output projection (transposed) ------------
    outp = ctx.enter_context(tc.tile_pool(name="outp", bufs=4))
    for sc in range(4):
        for mo in range(8):
            po = ps_mm.tile([P, 512], F32, name=f"po_{sc}_{mo}", tag="mm")
            for c in range(2):
                nc.tensor.matmul(
                    po[:],
                    lhsT=wo_sb[:, c, P * mo : P * (mo + 1)],
                    rhs=ctxt[c][sc][:],
                    start=(c == 0),
                    stop=(c == 1),
                )
            ob = outp.tile([P, 512], F32, tag="ob")
            if mo % 2 == 0:
                nc.scalar.copy(ob[:], po[:])
            else:
                nc.vector.tensor_copy(ob[:], po[:])
            nc.sync.dma_start(
                d["outT"][P * mo : P * (mo + 1), 512 * sc : 512 * (sc + 1)], ob[:]
            )


def build_program():
    nc = bacc.Bacc("TRN2", target_bir_lowering=False, debug=False, num_devices=8)
    d = {}
    d["xs"] = nc.dram_tensor("xs", [S, DM], F32, kind="ExternalInput").ap()
    d["wqkv"] = nc.dram_tensor("wqkv", [DM, 384], BF16, kind="ExternalInput").ap()
    d["wo"] = nc.dram_tensor("wo", [256, DM], BF16, kind="ExternalInput").ap()
    d["cos"] = nc.dram_tensor("cos", [S, 32], F32, kind="ExternalInput").ap()
    d["sin"] = nc.dram_tensor("sin", [S, 32], F32, kind="ExternalInput").ap()
    d["band"] = nc.dram_tensor("band", [NT, P, 3, P], BF16, kind="ExternalInput").ap()
    d["glob"] = nc.dram_tensor("glob", [NT, 4, P], BF16, kind="ExternalInput").ap()
    d["outT"] = nc.dram_tensor("outT", [DM, S], F32, kind="ExternalOutput").ap()
    with tile.TileContext(nc) as tc, ExitStack() as ctx:
        _build_kernel(ctx, tc, d)
    nc.compile()
    return nc


def make_masks(mask_np):
    """Pack the combined (caller mask & sliding-window|global) mask into the
    banded [k, q]-oriented tiles the kernel consumes."""
    mask_np = np.asarray(mask_np).astype(bool)
    q = np.arange(S)[:, None]
    k = np.arange(S)[None, :]
    wmask = ((k <= q) & (k > q - WINDOW)) | (k < NGLOB)
    combT = (mask_np[0, 0] & wmask).T.astype(np.float32)  # [k, q]
    band = np.zeros((NT, P, 3, P), np.float32)
    glob = np.zeros((NT, 4, P), np.float32)
    for t in range(NT):
        for kt in range(max(0, t - 2), t + 1):
            j = kt - (t - 2)
            band[t, :, j, :] = combT[P * kt : P * (kt + 1), P * t : P * (t + 1)]
        if t >= 3:
            glob[t] = combT[0:NGLOB, P * t : P * (t + 1)]
    return band, glob


def make_in_maps(x, cos, sin, mask, Wq, Wk, Wv, Wo):
    import ml_dtypes

    bf = ml_dtypes.bfloat16
    x, cos, sin = (np.asarray(a, np.float32) for a in (x, cos, sin))
    Wq, Wk, Wv, Wo = (np.asarray(a, np.float32).astype(bf) for a in (Wq, Wk, Wv, Wo))
    band, glob = make_masks(mask)
    band, glob = band.astype(bf), glob.astype(bf)
    in_maps = []
    for c in range(8):
        b, g = divmod(c, 4)
        wqkv = np.concatenate(
            [
                Wq[:, 256 * g : 256 * (g + 1)],
                Wk[:, 64 * g : 64 * (g + 1)],
                Wv[:, 64 * g : 64 * (g + 1)],
            ],
            axis=1,
        )
        in_maps.append(
            {
                "xs": np.ascontiguousarray(x[b]),
                "wqkv": np.ascontiguousarray(wqkv),
                "wo": np.ascontiguousarray(Wo[256 * g : 256 * (g + 1), :]),
                "cos": np.ascontiguousarray(cos),
                "sin": np.ascontiguousarray(sin),
                "band": band,
                "glob": glob,
            }
        )
    return in_maps


_PROGRAM = None


def _get_program():
    global _PROGRAM
    if _PROGRAM is None:
        _PROGRAM = build_program()
    return _PROGRAM


def kernel(x, cos, sin, mask, Wq, Wk, Wv, Wo, _trace=False, _trace_kwargs=None):
    nc = _get_program()
    in_maps = make_in_maps(x, cos, sin, mask, Wq, Wk, Wv, Wo)
    res = run_bass_kernel_spmd(
        nc, in_maps, list(range(8)), trace=_trace, **(_trace_kwargs or {})
    )
    out = np.zeros((B, S, DM), np.float32)
    for c in range(8):
        out[c // 4] += res.results[c]["outT"].T
    if _trace:
        kernel._last_results = res
    return out



# revision 2
# speedup vs baseline: 1.0076x; 1.0076x over previous
"""Trainium2 Bass kernel for GroupedQueryAttention (sliding-window + global).

Sharding v3: 8 cores = 2 (batch) x 4 (sequence chunks of 512 rows). Each core
computes ALL 16 heads for its 512 query rows, so outputs are disjoint row
blocks (no partial sums, no collectives). Host supplies transposed bf16
activations (xT). q/k transposes run on the PE (identity matmul) — DMA
dispatch on the sync engine is ~1.25us each and serializes. Masks (caller
mask & window | global) are baked on host into per-slot band masks + global
masks. Activation-table discipline: phase A uses {square, sqrt, copy} (one
table set), phase B uses {exp, copy}; with all phase-A Act ops queued before
phase-B ones, the table loads exactly twice.

Per-core layout: 7 context tiles ct=0..6: ct0 = abs tile 0 (global k/v
source), ct1..ct6 = abs tiles 4j-2 .. 4j+3 (zero-padded below 0), own q
tiles are ct3..ct6. Band k-slots for own tile t (ct=t+3): ct t+1, t+2, t+3.
Host permutes Wq cols / Wo rows so group g's 4 q-heads live at partition
half g%2 of qt pair-blocks 4*(g//2)+i, matching the kv head's half in kt
(matmul operands must share a base partition).
"""

import os
import sys

for _p in (
    "/opt/trn_rl_repo",
    "/root/.axon_site",
    "/root/.axon_site/_ro/pypackages",
    "/root/.axon_site/_ro/trn_rl_repo",
):
    if _p not in sys.path:
        sys.path.insert(0, _p)

from contextlib import ExitStack

import numpy as np

import concourse.bass as bass  # noqa: F401
import concourse.tile as tile
from concourse import bacc, mybir
from concourse.bass_utils import run_bass_kernel_spmd
from concourse.masks import make_identity

B, S, DM = 2, 2048, 1024
NH, NKV, DH = 16, 4, 64
WINDOW, NGLOB = 256, 4
SCALE = 1.0 / np.sqrt(DH)
P = 128
NCT = 7
NOWN = 4
F32 = mybir.dt.float32
BF16 = mybir.dt.bfloat16
MULT = mybir.AluOpType.mult
EXP = mybir.ActivationFunctionType.Exp
SQRT = mybir.ActivationFunctionType.Sqrt

DUMP = bool(int(os.environ.get("K2_DUMP", "0")))
_dump_specs = []


def _build_kernel(ctx, tc, d):
    nc = tc.nc

    def probe(tag, ap):
        if DUMP:
            if ("dump_" + tag) not in d:
                d["dump_" + tag] = nc.dram_tensor(
                    "dump_" + tag, list(ap.shape), ap.dtype, kind="ExternalOutput"
                ).ap()
                _dump_specs.append(tag)
            nc.sync.dma_start(d["dump_" + tag], ap)

    consts = ctx.enter_context(tc.tile_pool(name="consts", bufs=1))
    # split input loads so the proj pipeline starts as soon as its piece lands
    wall_sb = consts.tile([P, 8, 2560], BF16)  # [Wq | Wkv | Wo] col-concat
    wr = d["wall"].rearrange("(c p) n -> p c n", p=P)
    xt_sb = consts.tile([P, 8, NCT, P], BF16)
    xr = d["xT"].rearrange("(c p) t n -> p c t n", p=P)
    nc.sync.dma_start(wall_sb[:, :, 1024:1536], wr[:, :, 1024:1536])  # wkv first
    nc.sync.dma_start(xt_sb[:, :, 0:3, :], xr[:, :, 0:3, :])
    nc.sync.dma_start(wall_sb[:, :, 0:1024], wr[:, :, 0:1024])  # wq
    nc.sync.dma_start(xt_sb[:, :, 3:NCT, :], xr[:, :, 3:NCT, :])
    cs_sb = consts.tile([P, NCT, 64], F32)  # [cos | sin]
    nc.sync.dma_start(cs_sb[:], d["cs"])
    band_sb = consts.tile([P, NOWN, 3, P], BF16)  # bias: 0 pass / -1e9 block
    nc.sync.dma_start(band_sb[:], d["band"])
    glob_sb = consts.tile([4, NOWN, P], BF16)  # bias: 0 pass / -1e9 block
    nc.sync.dma_start(glob_sb[:], d["glob"])
    nc.sync.dma_start(wall_sb[:, :, 1536:2560], wr[:, :, 1536:2560])  # wo last
    eps_t = consts.tile([P, 1], F32)
    nc.vector.memset(eps_t[:], 1e-20)
    ident = consts.tile([P, P], F32)
    make_identity(nc, ident[:])
    ident_bf = consts.tile([P, P], BF16)
    nc.vector.tensor_copy(ident_bf[:], ident[:])

    wq = wall_sb[:, :, 0:1024]
    wkv = wall_sb[:, :, 1024:1536]
    wo = wall_sb[:, :, 1536:2560]

    big = ctx.enter_context(tc.tile_pool(name="big", bufs=1))
    qt_all = big.tile([P, NOWN, 8, P], BF16)  # [d2, t, pair m, s]
    kt_all = big.tile([P, NCT, 2, P], BF16)  # [d2, ct, kv pair u, s]
    vt = []
    for ct in range(NCT):
        v = big.tile([P, NKV, 65], BF16, name=f"v_{ct}", tag=f"v_{ct}")
        nc.vector.memset(v[:, :, 64:65], 1.0)
        vt.append(v)
    ctxt = big.tile([P, 8, 512], BF16, name="ctx_all", tag="ctx_all")

    work = ctx.enter_context(tc.tile_pool(name="work", bufs=2))
    attn = ctx.enter_context(tc.tile_pool(name="attn", bufs=4))
    outp = ctx.enter_context(tc.tile_pool(name="outp", bufs=2))

    ps_a = ctx.enter_context(tc.tile_pool(name="ps_a", bufs=3, space="PSUM"))
    ps_tr = ctx.enter_context(tc.tile_pool(name="ps_tr", bufs=1, space="PSUM"))
    ps_sc = ctx.enter_context(tc.tile_pool(name="ps_sc", bufs=2, space="PSUM"))
    ps_cx = ctx.enter_context(tc.tile_pool(name="ps_cx", bufs=2, space="PSUM"))

    rp_tiles = {}

    def phase_a(ct):
        """Project q/k/v, L2-normalize (recip(sqrt)), RoPE."""
        own = ct >= 3
        G = 20 if own else NKV  # groups: [0:4]=kv heads, [4:20]=q heads

        pkv = ps_a.tile([P, 512], F32, name=f"pkv_{ct}", tag="a")
        for c in range(8):
            nc.tensor.matmul(
                pkv[:], lhsT=xt_sb[:, c, ct, :], rhs=wkv[:, c, :],
                start=(c == 0), stop=(c == 7),
            )
        pq = []
        if own:
            for h in range(2):
                pqh = ps_a.tile([P, 512], F32, name=f"pq_{ct}_{h}", tag="a")
                for c in range(8):
                    nc.tensor.matmul(
                        pqh[:], lhsT=xt_sb[:, c, ct, :],
                        rhs=wq[:, c, 512 * h : 512 * (h + 1)],
                        start=(c == 0), stop=(c == 7),
                    )
                pq.append(pqh)

        # sum of squares per 64-dim group -> 1/sqrt on Act(sqrt)+DVE(recip)
        sq = work.tile([P, 1280], BF16, tag="sq")
        nsq = work.tile([P, 20], F32, tag="nsq")
        nc.scalar.square(sq[:, 0:256], pkv[:, 0:256])
        if own:
            nc.scalar.square(sq[:, 256:768], pq[0][:])
            nc.scalar.square(sq[:, 768:1280], pq[1][:])
        nc.vector.tensor_reduce(
            nsq[:, 0:G].rearrange("p (g o) -> p g o", g=G),
            sq[:, 0 : 64 * G].rearrange("p (g n) -> p g n", g=G),
            axis=mybir.AxisListType.X,
            op=mybir.AluOpType.add,
        )
        nrm = work.tile([P, 20], F32, tag="nrm")
        nc.scalar.activation(nrm[:, 0:G], nsq[:, 0:G], SQRT, bias=eps_t[:])
        rcn = work.tile([P, 20], F32, tag="rcn")
        nc.vector.reciprocal_approx_fast(rcn[:, 0:G], nrm[:, 0:G])

        # normalize into qkn layout [kv 0:256 | q 256:1280] (bf16: 2x DVE)
        qkn = work.tile([P, 1280], BF16, tag="qkn")
        nc.vector.tensor_tensor(
            qkn[:, 0:256].rearrange("p (g n) -> p g n", g=4),
            pkv[:, 0:256].rearrange("p (g n) -> p g n", g=4),
            rcn[:, 0:4].unsqueeze(-1).broadcast_to([P, 4, DH]),
            op=MULT,
        )
        if own:
            for h in range(2):
                nc.vector.tensor_tensor(
                    qkn[:, 256 + 512 * h : 768 + 512 * h].rearrange(
                        "p (g n) -> p g n", g=8
                    ),
                    pq[h][:].rearrange("p (g n) -> p g n", g=8),
                    rcn[:, 4 + 8 * h : 12 + 8 * h].unsqueeze(-1).broadcast_to([P, 8, DH]),
                    op=MULT,
                )

        # RoPE: halves (0:32, 32:64) of each 64-dim group; DVE/GpSimd split
        qv = qkn[:, 0 : 64 * G].rearrange("p (g n) -> p g n", g=G)
        x1, x2 = qv[:, :, 0:32], qv[:, :, 32:64]
        cb = cs_sb[:, ct, 0:32].unsqueeze(1).broadcast_to([P, G, 32])
        sb = cs_sb[:, ct, 32:64].unsqueeze(1).broadcast_to([P, G, 32])
        rp = work.tile([P, 1280], BF16, tag="rp")
        rv = rp[:, 0 : 64 * G].rearrange("p (g n) -> p g n", g=G)
        ta = work.tile([P, 640], BF16, tag="ta")
        tb = work.tile([P, 640], BF16, tag="tb")
        ta2 = work.tile([P, 640], BF16, tag="ta2")
        tb2 = work.tile([P, 640], BF16, tag="tb2")
        tav = ta[:, 0 : 32 * G].rearrange("p (g n) -> p g n", g=G)
        tbv = tb[:, 0 : 32 * G].rearrange("p (g n) -> p g n", g=G)
        ta2v = ta2[:, 0 : 32 * G].rearrange("p (g n) -> p g n", g=G)
        tb2v = tb2[:, 0 : 32 * G].rearrange("p (g n) -> p g n", g=G)
        nc.vector.tensor_tensor(tav, x1, cb, op=MULT)
        nc.gpsimd.tensor_tensor(tbv, x2, sb, op=MULT)
        nc.gpsimd.tensor_tensor(ta2v, x1, sb, op=MULT)
        nc.vector.tensor_tensor(tb2v, x2, cb, op=MULT)
        nc.vector.tensor_sub(rv[:, :, 0:32], tav, tbv)
        nc.gpsimd.tensor_add(rv[:, :, 32:64], ta2v, tb2v)
        rp_tiles[ct] = rp

        # v copy (kv cols 256:512 of pkv)
        nc.scalar.copy(
            vt[ct][:, :, 0:64], pkv[:, 256:512].rearrange("p (g n) -> p g n", g=4)
        )

    def phase_t(ct):
        """Transpose rope'd q/k pair-blocks on the PE; copy into qt/kt."""
        own = ct >= 3
        t = ct - 3
        rp = rp_tiles[ct]
        rounds = []
        if own:
            rounds = [(256, [0, 1, 2, 3]), (768, [4, 5, 6, 7])]
        ktr = (0, [0, 1])
        for base, ms in rounds:
            tr = ps_tr.tile([P, 4, P], BF16, name=f"trq_{ct}_{base}", tag="tr")
            for i, m in enumerate(ms):
                nc.tensor.transpose(
                    tr[:, i, :], rp[:, base + P * (m % 4) : base + P * (m % 4) + P],
                    ident_bf[:],
                )
            for i, m in enumerate(ms):
                eng = nc.vector if i % 2 == 0 else nc.scalar
                if eng is nc.scalar:
                    nc.scalar.copy(qt_all[:, t, m, :], tr[:, i, :])
                else:
                    nc.vector.tensor_copy(qt_all[:, t, m, :], tr[:, i, :])
        base, us = ktr
        tr = ps_tr.tile([P, 4, P], BF16, name=f"trk_{ct}", tag="tr")
        for u in us:
            nc.tensor.transpose(tr[:, u, :], rp[:, P * u : P * (u + 1)], ident_bf[:])
        nc.vector.tensor_copy(kt_all[:, ct, 0, :], tr[:, 0, :])
        nc.scalar.copy(kt_all[:, ct, 1, :], tr[:, 1, :])

    def phase_b(t):
        """Banded + global attention for own tile t (ct = t+3).

        Band masks for slots 0/1 are -1e9 biases accumulated into the scores
        psum by an identity matmul (keeps the pre-exp chain on the PE); the
        diagonal slot uses a causal affine_select on GpSimd; global keys are
        batched across all 4 groups (one exp + one mask-mult for 16 rows).
        """
        for g in range(4):
            u, hf = g // 2, g % 2
            hb = 64 * hf
            m0 = 4 * (g // 2)
            qrhs = qt_all[hb : hb + 64, t, m0 : m0 + 4, :]  # [64, 4, 128]
            pcx = ps_cx.tile([65, 512], F32, name=f"pcx_{t}_{g}", tag="cx")

            psg = ps_sc.tile([4, 512], F32, name=f"psg_{t}_{g}", tag="sc")
            nc.tensor.matmul(
                psg[:], lhsT=kt_all[hb : hb + 64, 0, u, 0:4], rhs=qrhs,
                start=True, stop=False,
            )
            nc.tensor.matmul(
                psg[:].rearrange("p (h q) -> p h q", h=4),
                lhsT=ident_bf[0:4, 0:4],
                rhs=glob_sb[0:4, t, :].unsqueeze(1).broadcast_to([4, 4, P]),
                start=False, stop=True,
            )
            exg = attn.tile([4, 512], BF16, tag="exg")
            nc.scalar.activation(exg[:], psg[:], EXP, scale=SCALE)

            exs = []
            for j3 in range(3):
                ct_k = t + j3 + 1
                ps = ps_sc.tile([P, 512], F32, name=f"ps_{t}_{g}_{j3}", tag="sc")
                nc.tensor.matmul(
                    ps[:], lhsT=kt_all[hb : hb + 64, ct_k, u, :], rhs=qrhs,
                    start=True, stop=False,
                )
                nc.tensor.matmul(
                    ps[:].rearrange("p (h q) -> p h q", h=4),
                    lhsT=ident_bf[:],
                    rhs=band_sb[:, t, j3, :].unsqueeze(1).broadcast_to([P, 4, P]),
                    start=False, stop=True,
                )
                ex = attn.tile([P, 512], BF16, tag="ex")
                nc.scalar.activation(ex[:], ps[:], EXP, scale=SCALE)
                exs.append(ex)
            for j3 in range(3):
                nc.tensor.matmul(
                    pcx[:], lhsT=vt[t + j3 + 1][:, g, :], rhs=exs[j3][:],
                    start=(j3 == 0), stop=False,
                )
            nc.tensor.matmul(
                pcx[:], lhsT=vt[0][0:4, g, :], rhs=exg[:], start=False, stop=True
            )

            # softmax denominators: PSUM row -> SBUF, approx recip (DVE),
            # partition-broadcast (GpSimd), fused normalize+stack (one DVE op)
            sm = attn.tile([1, 512], F32, tag="sm")
            nc.vector.tensor_copy(sm[:], pcx[64:65, :])
            rc1 = attn.tile([1, 512], F32, tag="rc1")
            nc.vector.reciprocal_approx_fast(rc1[:], sm[:])
            rc = attn.tile([64, 512], F32, tag="rc")
            nc.gpsimd.partition_broadcast(rc[:], rc1[:])
            if DUMP and t == 0:
                probe(f"rc1_{t}_{g}", rc1[:])
            nc.vector.tensor_tensor(
                ctxt[hb : hb + 64, m0 : m0 + 4, P * t : P * (t + 1)],
                pcx[0:64, :].rearrange("p (h q) -> p h q", h=4),
                rc[:].rearrange("p (h q) -> p h q", h=4),
                op=MULT,
            )

    def phase_c(t):
        """Output projection for own tile t: out rows [128t:128t+128]."""
        ob = outp.tile([P, 1024], F32, tag="ob")
        for h in range(2):
            pw = ps_sc.tile([P, 512], F32, name=f"pw_{t}_{h}", tag="sc")
            for m in range(8):
                nc.tensor.matmul(
                    pw[:],
                    lhsT=ctxt[:, m, P * t : P * (t + 1)],
                    rhs=wo[:, m, 512 * h : 512 * (h + 1)],
                    start=(m == 0), stop=(m == 7),
                )
            if h == 0:
                nc.vector.tensor_copy(ob[:, 0:512], pw[:])
            else:
                nc.scalar.copy(ob[:, 512:1024], pw[:])
            nc.sync.dma_start(
                d["out"][P * t : P * (t + 1), 512 * h : 512 * (h + 1)],
                ob[:, 512 * h : 512 * (h + 1)],
            )
        if DUMP:
            probe(f"ob_{t}", ob[:, 0:64])

    # program order IS the per-engine schedule: all A (transposes trail one
    # tile so the PE alternates proj/transpose), then B/C
    phase_a(0)
    phase_a(1)
    phase_t(0)
    phase_a(2)
    phase_t(1)
    phase_a(3)
    phase_t(2)
    phase_a(4)
    phase_t(3)
    phase_a(5)
    phase_t(4)
    phase_a(6)
    phase_t(5)
    phase_t(6)
    phase_b(0)
    phase_b(1)
    phase_c(0)
    phase_b(2)
    phase_c(1)
    phase_b(3)
    phase_c(2)
    phase_c(3)


def build_program():
    nc = bacc.Bacc("TRN2", target_bir_lowering=False, debug=False, num_devices=8)
    d = {}
    d["xT"] = nc.dram_tensor("xT", [DM, NCT, P], BF16, kind="ExternalInput").ap()
    d["wall"] = nc.dram_tensor("wall", [DM, 2560], BF16, kind="ExternalInput").ap()
    d["cs"] = nc.dram_tensor("cs", [P, NCT, 64], F32, kind="ExternalInput").ap()
    d["band"] = nc.dram_tensor("band", [P, NOWN, 3, P], BF16, kind="ExternalInput").ap()
    d["glob"] = nc.dram_tensor("glob", [4, NOWN, P], BF16, kind="ExternalInput").ap()
    d["out"] = nc.dram_tensor("out", [512, DM], F32, kind="ExternalOutput").ap()
    with tile.TileContext(nc) as tc, ExitStack() as ctx:
        _build_kernel(ctx, tc, d)
    nc.compile()
    return nc


def make_in_maps(x, cos, sin, mask, Wq, Wk, Wv, Wo):
    import ml_dtypes

    bf = ml_dtypes.bfloat16
    x = np.asarray(x, np.float32)
    cos = np.asarray(cos, np.float32)
    sin = np.asarray(sin, np.float32)
    mask_np = np.asarray(mask).astype(bool)[0, 0]
    Wq, Wk, Wv, Wo = (np.asarray(a, np.float32).astype(bf) for a in (Wq, Wk, Wv, Wo))
    wkv = np.concatenate([Wk, Wv], axis=1)

    # head permutation: pair-tile m holds head 8*(m//4)+m%4 (half 0) and
    # that +4 (half 1), so group g's heads sit at partition half g%2
    head_perm = []
    for m in range(8):
        head_perm += [8 * (m // 4) + m % 4, 8 * (m // 4) + m % 4 + 4]
    dim_perm = np.concatenate([np.arange(64 * h, 64 * (h + 1)) for h in head_perm])
    wall = np.ascontiguousarray(
        np.concatenate([Wq[:, dim_perm], wkv, Wo[dim_perm, :]], axis=1)
    )

    qi = np.arange(S)[:, None]
    ki = np.arange(S)[None, :]
    wmask = ((ki <= qi) & (ki > qi - WINDOW)) | (ki < NGLOB)
    combT = (mask_np & wmask).astype(np.float32).T  # [k, q]

    xT = [np.ascontiguousarray(x[b].T.astype(bf)) for b in range(B)]

    in_maps = []
    for c in range(8):
        b, j = divmod(c, 4)
        abs_tiles = [0] + [4 * j - 2 + i for i in range(6)]
        xt_core = np.zeros((DM, NCT, P), bf)
        cs_core = np.zeros((P, NCT, 64), np.float32)
        for i, at in enumerate(abs_tiles):
            if at < 0:
                continue
            xt_core[:, i, :] = xT[b][:, P * at : P * (at + 1)]
            cs_core[:, i, 0:32] = cos[P * at : P * (at + 1), :]
            cs_core[:, i, 32:64] = sin[P * at : P * (at + 1), :]
        # additive bias masks: 0 = pass, -1e9 = blocked
        band = np.full((P, NOWN, 3, P), -1e9, np.float32)
        glob = np.full((4, NOWN, P), -1e9, np.float32)
        for t in range(NOWN):
            tau = 4 * j + t
            qs = slice(P * tau, P * (tau + 1))
            band_kts = []
            for j3 in range(3):
                at = tau - 2 + j3
                if at >= 0:
                    band[:, t, j3, :] = (combT[P * at : P * (at + 1), qs] - 1.0) * 1e9
                    band_kts.append(at)
            if 0 not in band_kts:
                glob[:, t, :] = (combT[0:NGLOB, qs] - 1.0) * 1e9
        in_maps.append(
            {
                "xT": xt_core,
                "wall": wall,
                "cs": cs_core,
                "band": band.astype(bf),
                "glob": glob.astype(bf),
            }
        )
    return in_maps


_PROGRAM = None


def _get_program():
    global _PROGRAM
    if _PROGRAM is None:
        _PROGRAM = build_program()
    return _PROGRAM


def kernel(x, cos, sin, mask, Wq, Wk, Wv, Wo, _trace=False, _trace_kwargs=None):
    nc = _get_program()
    in_maps = make_in_maps(x, cos, sin, mask, Wq, Wk, Wv, Wo)
    res = run_bass_kernel_spmd(
        nc, in_maps, list(range(8)), trace=_trace, **(_trace_kwargs or {})
    )
    out = np.zeros((B, S, DM), np.float32)
    for c in range(8):
        b, j = divmod(c, 4)
        out[b, 512 * j : 512 * (j + 1), :] = res.results[c]["out"]
    if _trace:
        kernel._last_results = res
    return out
